# revision 24
# baseline (speedup 1.0000x reference)
"""Trainium2 Bass kernel for nn_GPSLayer (GCNConv + TransformerConv + FFN).

Strategy (v2):
  - Node phase SPLIT across the 8 cores: core c computes [dinv*x@Wgcn | x@Wk+bk
    | x@Wv+bv] for its 1/8 node slice only, then an AllGather (Shared output)
    assembles the full [NR2, 3D] gather-source in HBM.  q = x@Wq+bq and
    x_skip = x@Wskip+bskip stay core-local in SBUF (q in bf16).
  - Edge phase: dst nodes sharded by contiguous 128-ranges; per dst-tile,
    dma_gather the [E_t, 384] payload rows (lo/hi halves for int16 indexing;
    padding slots use idx=-1 which the Q7 ucode strips for free).
  - q[dst] per edge is NOT gathered: qg = stT^T @ q_tile on the PE, where
    stT (one-hot, bf16) is host-precomputed and DMA'd.  st (f32 one-hot for
    the segment-sum matmuls) is built on-chip with one broadcast is_eq.
  - Segment-sum via one-hot matmuls into PSUM (pa=gcn, pb=[attn*v|sum exp]).
  - Post phase (softmax normalize, beta-gate, LN, FFN, LN) runs batched over
    chunks of 13 dst tiles with [128, chunk*128] vector ops.
"""

import math
import os
from contextlib import ExitStack

import numpy as np
import ml_dtypes

import concourse.bacc as bacc
import concourse.bass as bass
import concourse.mybir as mybir
import concourse.tile as tile
from concourse.bass_utils import run_bass_kernel_spmd
from concourse.library_config import mlp as MLP_LIB

F32 = mybir.dt.float32
BF16 = mybir.dt.bfloat16
I16 = mybir.dt.int16
AX = mybir.AxisListType
OP = mybir.AluOpType
AF = mybir.ActivationFunctionType

N = 50000
D = 128
H = 4
C = D // H
P = 128
NCORES = 8
TPC = 49                      # dst/node tiles per core
CORE_NODES = TPC * P          # 6272
NR2 = NCORES * CORE_NODES     # 50176 padded rows
SPLIT = 32768
EPS = 1e-5
PAD_DL = 200.0
CHUNK = 13                    # post-phase batch width (dst tiles)

LAST_EXEC_NS = None
LAST_RESULTS = None
LAST_NC = None


def preprocess(x, edge_index):
    """Host-side: shard+sort edges, build gather indices (-1 padding), st^T
    one-hot (bf16), per-core transposed x slices and degree columns."""
    src = np.asarray(edge_index[0]).astype(np.int64)
    dst = np.asarray(edge_index[1]).astype(np.int64)
    E = src.shape[0]
    deg = np.bincount(dst, minlength=N).astype(np.float32)

    core = dst // CORE_NODES
    tl = (dst % CORE_NODES) // P
    dl = (dst % P).astype(np.int64)
    hi = (src >= SPLIT).astype(np.int64)

    key = (core * TPC + tl) * 2 + hi
    ngroups = NCORES * TPC * 2
    counts = np.bincount(key, minlength=ngroups).reshape(NCORES, TPC, 2)

    # uniform-across-cores block counts per (tile, half) — SPMD program needs it
    NB_LO = np.maximum(1, -(-counts[:, :, 0].max(axis=0) // P)).astype(np.int64)
    NB_HI = (-(-counts[:, :, 1].max(axis=0) // P)).astype(np.int64)
    NBT = NB_LO + NB_HI
    OFFB = np.concatenate([[0], np.cumsum(NBT)])[:-1]
    TB = int(NBT.sum())

    lo_start = OFFB * P
    hi_start = (OFFB + NB_LO) * P
    group_base = np.stack([lo_start, hi_start], axis=1)  # [TPC, 2]

    order = np.argsort(key, kind="stable")
    group_start_idx = np.concatenate([[0], np.cumsum(counts.reshape(-1))])[:-1]
    rank = np.empty(E, dtype=np.int64)
    rank[order] = np.arange(E) - group_start_idx[key[order]]

    pos = group_base[tl, hi] + rank
    srcv = np.where(hi == 1, src - SPLIT, src).astype(np.int16)

    # pad slots gather row 0 (the deployed Q7 ucode mishandles -1 sentinels)
    idx_all = np.zeros((NCORES, TB * P), dtype=np.int16)
    dst_all = np.full((NCORES, TB * P), PAD_DL, dtype=np.float32)
    flat = core * (TB * P) + pos
    idx_all.reshape(-1)[flat] = srcv
    dst_all.reshape(-1)[flat] = dl.astype(np.float32)

    # wrapped int16 index layout [16, TB*8] replicated to 128 partitions
    idx_wrap = np.ascontiguousarray(idx_all.reshape(NCORES, TB * 8, 16).transpose(0, 2, 1))
    idx_rep = np.tile(idx_wrap, (1, 8, 1))                       # [NCORES, 128, TB*8]
    dst_col = np.ascontiguousarray(dst_all.reshape(NCORES, TB, P).transpose(0, 2, 1))

    # stT one-hot bf16: stT[c, j, b*128+e] = (dst_all[c, b*128+e] == j)
    jj = np.arange(P, dtype=np.float32)[:, None]
    stT = np.empty((NCORES, P, TB * P), dtype=ml_dtypes.bfloat16)
    for c in range(NCORES):
        stT[c] = (dst_all[c][None, :] == jj).astype(ml_dtypes.bfloat16)

    # per-core transposed x slice + degree column
    x = np.asarray(x, dtype=np.float32)
    x_pad = np.zeros((NR2, D), dtype=np.float32)
    x_pad[:N] = x
    deg_pad = np.ones(NR2, dtype=np.float32)
    deg_pad[:N] = deg
    xT = np.empty((NCORES, D, CORE_NODES), dtype=np.float32)
    dego = np.empty((NCORES, P, TPC), dtype=np.float32)
    for c in range(NCORES):
        a, b = c * CORE_NODES, (c + 1) * CORE_NODES
        xT[c] = np.ascontiguousarray(x_pad[a:b].T)
        dego[c] = np.ascontiguousarray(deg_pad[a:b].reshape(TPC, P).T)

    meta = dict(NB_LO=NB_LO.tolist(), NB_HI=NB_HI.tolist(), OFFB=OFFB.tolist(),
                TB=TB, NBT_MAX=int(NBT.max()))
    percore = dict(idx_rep=idx_rep, dst_col=dst_col, stT=stT, xT=xT, dego=dego)
    return meta, percore


def build_program(meta, gw_scalar):
    NB_LO, NB_HI, OFFB = meta["NB_LO"], meta["NB_HI"], meta["OFFB"]
    TB, NBM = meta["TB"], meta["NBT_MAX"]

    nc = bacc.Bacc("TRN2", target_bir_lowering=False, debug=False, num_devices=NCORES)

    def inp(name, shape, dt=F32):
        return nc.dram_tensor(name, shape, dt, kind="ExternalInput").ap()

    xT_d = inp("xT", [D, CORE_NODES])
    dego_d = inp("dego", [P, TPC])
    wall_d = inp("W_all", [D, 3 * D])
    wqs_d = inp("wq_skip", [D, 2 * D])
    bcat_d = inp("b_cat_rep", [P, 2 * D])
    bq_d = inp("bq_rep", [P, D])
    bskip_d = inp("b_skip_rep", [P, D])
    bgcn_d = inp("b_gcn_rep", [P, D])
    g1_d = inp("g1_rep", [P, D])
    b1_d = inp("b1_rep", [P, D])
    g2_d = inp("g2_rep", [P, D])
    b2_d = inp("b2_rep", [P, D])
    wba_d = inp("wba_rep", [P, D])
    wbb_d = inp("wbb_rep", [P, D])
    wrel_d = inp("w_rel", [D, 2 * D])
    wr0_d = inp("w_root0", [D, D])
    wr1_d = inp("w_root1", [D, D])
    iot_d = inp("iota_rep", [P, P])
    idn_d = inp("ident", [P, P])
    idx_d = inp("idx_rep", [P, TB * 8], I16)
    sdl_d = inp("dst_col", [P, TB])
    stT_d = inp("stT", [P, TB * P], BF16)

    gpart = nc.dram_tensor("gpart", [CORE_NODES, 3 * D], F32).ap()
    gall = nc.dram_tensor("gall", [NR2, 3 * D], F32, addr_space="Shared").ap()
    out_d = nc.dram_tensor("out", [CORE_NODES, D], F32, kind="ExternalOutput").ap()

    inv_sqrt_c = 1.0 / math.sqrt(C)

    with tile.TileContext(nc) as tc, ExitStack() as ctx:
        lib_inst = nc.gpsimd.load_library(MLP_LIB)
        cp = ctx.enter_context(tc.tile_pool(name="const", bufs=1))

        def ctile(src_ap, shape, dt=F32, tag=None):
            t = cp.tile(shape, dt, tag=tag)
            nc.sync.dma_start(t[:], src_ap)
            return t

        wall = ctile(wall_d, [D, 3 * D], tag="wall")
        wqs = ctile(wqs_d, [D, 2 * D], tag="wqs")
        bcat = ctile(bcat_d, [P, 2 * D], tag="bcat")
        bq = ctile(bq_d, [P, D], tag="bq")
        bskip = ctile(bskip_d, [P, D], tag="bskip")
        bgcn = ctile(bgcn_d, [P, D], tag="bgcn")
        g1 = ctile(g1_d, [P, D], tag="g1")
        b1 = ctile(b1_d, [P, D], tag="b1")
        g2 = ctile(g2_d, [P, D], tag="g2")
        b2 = ctile(b2_d, [P, D], tag="b2")
        wba = ctile(wba_d, [P, D], tag="wba")
        wbb = ctile(wbb_d, [P, D], tag="wbb")
        wrel = ctile(wrel_d, [D, 2 * D], tag="wrel")
        wr0 = ctile(wr0_d, [D, D], tag="wr0")
        wr1 = ctile(wr1_d, [D, D], tag="wr1")
        iot = ctile(iot_d, [P, P], tag="iot")
        idn = ctile(idn_d, [P, P], tag="idn")
        sdl = ctile(sdl_d, [P, TB], tag="sdl")
        sdego = ctile(dego_d, [P, TPC], tag="sdego")

        dinvo = cp.tile([P, TPC], F32, tag="dinvo")
        with tc.tile_pool(name="setup", bufs=1) as sp:
            t1 = sp.tile([P, TPC], F32, tag="din1")
            t2 = sp.tile([P, TPC], F32, tag="din2")
            nc.vector.tensor_scalar_max(t1[:], sdego[:], 1.0)
            nc.scalar.activation(t2[:], t1[:], AF.Sqrt)
            nc.vector.reciprocal(t1[:], t2[:])
            nc.vector.tensor_scalar(t2[:], sdego[:], 0.0, None, OP.is_gt)
            nc.vector.tensor_mul(dinvo[:], t1[:], t2[:])

        pers = ctx.enter_context(tc.tile_pool(name="pers", bufs=1))
        qall = pers.tile([P, TPC, D], BF16, tag="qall")
        xrs = pers.tile([P, TPC, D], F32, tag="xrs")



        # ---------------- node phase (own 1/8 slice only) ----------------
        nscope = ExitStack() if os.environ.get("GPS_NOSCOPE", "0") != "1" else ctx
        with nscope if nscope is not ctx else ExitStack():
            npx = nscope.enter_context(tc.tile_pool(name="npx", bufs=3))
            npo = nscope.enter_context(tc.tile_pool(name="npo", bufs=3))
            ppsn = nscope.enter_context(tc.tile_pool(name="ppsn", bufs=2, space="PSUM"))
            for t in range(TPC):
                xt = npx.tile([D, P], F32, tag="xt")
                nc.sync.dma_start(xt[:], xT_d[:, t * P : (t + 1) * P])
                pm1 = ppsn.tile([P, 3 * D], F32, tag="pm1")
                nc.tensor.matmul(pm1[:], xt[:], wall[:], start=True, stop=True)
                pm2 = ppsn.tile([P, 2 * D], F32, tag="pm2")
                nc.tensor.matmul(pm2[:], xt[:], wqs[:], start=True, stop=True)
                go = npo.tile([P, 3 * D], F32, tag="go")
                nc.scalar.activation(go[:, 0:D], pm1[:, 0:D], AF.Copy,
                                     scale=dinvo[:, t : t + 1])
                nc.vector.tensor_add(go[:, D : 3 * D], pm1[:, D : 3 * D], bcat[:])
                nc.sync.dma_start(gpart[t * P : (t + 1) * P, :], go[:])
                nc.vector.tensor_add(qall[:, t, :], pm2[:, 0:D], bq[:])
                nc.vector.tensor_add(xrs[:, t, :], pm2[:, D : 2 * D], bskip[:])

        if os.environ.get("GPS_NOCC", "0") != "1":
            nc.gpsimd.collective_compute(
                "AllGather", OP.bypass,
                replica_groups=[list(range(NCORES))],
                ins=[gpart[:, :]], outs=[gall[:, :]],
            )

        STAGE = int(os.environ.get("GPS_STAGE", "9"))
        if STAGE == 1:
            with tc.tile_pool(name="dbg", bufs=1) as dbg:
                for r in range(4):
                    dt_ = dbg.tile([P, 3 * D], F32, tag="dbg")
                    nc.sync.dma_start(dt_[:], gall[r * 12288 : r * 12288 + P, :])
                    nc.sync.dma_start(out_d[r * P : (r + 1) * P, :], dt_[:, 0:D])

        # ---------------- edge phase ----------------
        epg = ctx.enter_context(tc.tile_pool(name="epg", bufs=3))
        epi = ctx.enter_context(tc.tile_pool(name="epi", bufs=3))
        eps = ctx.enter_context(tc.tile_pool(name="eps", bufs=2))
        epq = ctx.enter_context(tc.tile_pool(name="epq", bufs=1, space="PSUM"))
        epab = ctx.enter_context(tc.tile_pool(name="epab", bufs=2, space="PSUM"))
        chp = ctx.enter_context(tc.tile_pool(name="chp", bufs=2))
        chs = ctx.enter_context(tc.tile_pool(name="chs", bufs=2))
        ffp = ctx.enter_context(tc.tile_pool(name="ffp", bufs=1, space="PSUM"))
        ffs = ctx.enter_context(tc.tile_pool(name="ffs", bufs=2))

        # zero the gather buffers once: stripped (-1) pad slots leave stale
        # bytes; first rounds must not contain NaN bit patterns.
        for _ in range(3):
            z = epg.tile([P, NBM, 3 * D], F32, tag="gt")
            nc.vector.memset(z[:], 0.0)

        reg_cache = {}

        def nreg(val):
            if val not in reg_cache:
                r = nc.gpsimd.alloc_register(f"ni_{val}")
                nc.gpsimd.reg_mov(r, val)
                reg_cache[val] = r
            return reg_cache[val]

        def emit_post(c0, cw, Ach, Gch, HLch, Sch):
            Av = Ach[:, 0:cw, :]
            Gv = Gch[:, 0:cw, :]
            HLv = HLch[:, 0:cw, :]
            Sv = Sch[:, 0:cw, :]
            r4 = lambda ap: ap.rearrange("p t (h c) -> p t h c", h=H)
            bc1 = lambda t_: t_[:].unsqueeze(1).broadcast_to([P, cw, D])
            bc2 = lambda ap: ap.unsqueeze(2).broadcast_to([P, cw, D])

            if STAGE == 7:
                nc.sync.dma_start(
                    out_d[c0 * P : (c0 + cw) * P, :].rearrange("(t p) d -> p t d", p=P),
                    xrs[:, c0 : c0 + cw, :])
                return

            # softmax normalize + gcn degree scale
            nc.vector.tensor_scalar_max(Sv, Sv, 1e-16)
            nc.vector.reciprocal(Sv, Sv)
            nc.vector.tensor_tensor(
                out=r4(Av), in0=r4(Av),
                in1=Sv.unsqueeze(3).broadcast_to([P, cw, H, C]), op=OP.mult)
            if STAGE == 8:
                nc.sync.dma_start(
                    out_d[c0 * P : (c0 + cw) * P, :].rearrange("(t p) d -> p t d", p=P),
                    Av)
                return
            nc.vector.tensor_tensor(
                out=Gv, in0=Gv, in1=bc2(dinvo[:, c0 : c0 + cw]), op=OP.mult)
            nc.vector.tensor_tensor(out=Gv, in0=Gv, in1=bc1(bgcn), op=OP.add)
            if STAGE == 10:
                nc.sync.dma_start(
                    out_d[c0 * P : (c0 + cw) * P, :].rearrange("(t p) d -> p t d", p=P),
                    Gv)
                return

            # beta gate
            Xv = xrs[:, c0 : c0 + cw, :]
            r1 = chs.tile([P, CHUNK], F32, tag="r1")
            r2 = chs.tile([P, CHUNK], F32, tag="r2")
            nc.vector.tensor_tensor(out=HLv, in0=Av, in1=bc1(wba), op=OP.mult)
            nc.vector.tensor_reduce(r1[:, 0:cw], HLv, AX.X, OP.add)
            nc.vector.tensor_tensor(out=HLv, in0=Xv, in1=bc1(wbb), op=OP.mult)
            nc.vector.tensor_reduce(r2[:, 0:cw], HLv, AX.X, OP.add)
            nc.vector.tensor_add(r1[:, 0:cw], r1[:, 0:cw], r2[:, 0:cw])
            bet = chs.tile([P, CHUNK], F32, tag="bet")
            nc.scalar.activation(bet[:, 0:cw], r1[:, 0:cw], AF.Sigmoid)
            nc.vector.tensor_sub(HLv, Xv, Av)
            nc.vector.tensor_tensor(out=HLv, in0=HLv, in1=bc2(bet[:, 0:cw]), op=OP.mult)
            nc.vector.tensor_add(HLv, HLv, Av)
            if gw_scalar != 1.0:
                nc.vector.tensor_scalar_mul(HLv, HLv, float(gw_scalar))
            nc.vector.tensor_add(Gv, Gv, HLv)      # h
            if STAGE == 5:
                nc.sync.dma_start(
                    out_d[c0 * P : (c0 + cw) * P, :].rearrange("(t p) d -> p t d", p=P),
                    Gv)
                return

            # LN1  (LN(2h) == LN(h))
            mu = chs.tile([P, CHUNK], F32, tag="mu")
            nc.vector.tensor_reduce(mu[:, 0:cw], Gv, AX.X, OP.add)
            nc.vector.tensor_scalar_mul(mu[:, 0:cw], mu[:, 0:cw], 1.0 / D)
            nc.vector.tensor_tensor(out=Gv, in0=Gv, in1=bc2(mu[:, 0:cw]), op=OP.subtract)
            nc.vector.tensor_mul(HLv, Gv, Gv)
            vv = chs.tile([P, CHUNK], F32, tag="vv")
            nc.vector.tensor_reduce(vv[:, 0:cw], HLv, AX.X, OP.add)
            nc.vector.tensor_scalar(vv[:, 0:cw], vv[:, 0:cw], 1.0 / D, EPS,
                                    OP.mult, OP.add)
            sd = chs.tile([P, CHUNK], F32, tag="sd")
            nc.scalar.activation(sd[:, 0:cw], vv[:, 0:cw], AF.Sqrt)
            nc.vector.reciprocal(sd[:, 0:cw], sd[:, 0:cw])
            nc.vector.tensor_tensor(out=HLv, in0=Gv, in1=bc2(sd[:, 0:cw]), op=OP.mult)
            nc.vector.tensor_tensor(out=HLv, in0=HLv, in1=bc1(g1), op=OP.mult)
            nc.vector.tensor_tensor(out=HLv, in0=HLv, in1=bc1(b1), op=OP.add)  # hl
            if STAGE == 6:
                nc.sync.dma_start(
                    out_d[c0 * P : (c0 + cw) * P, :].rearrange("(t p) d -> p t d", p=P),
                    HLv)
                return

            # FFN per tile; o2 overwrites Gch (xc dead).  pT/o1/o2 packed into
            # one PSUM bank: [0:D]=pT, [D:3D]=o1, [3D:4D]=o2.
            for i in range(cw):
                fb = ffp.tile([P, 4 * D], F32, tag="fb")
                nc.tensor.transpose(fb[:, 0:D], HLch[:, i, :], idn[:])
                hT = ffs.tile([P, D], F32, tag="hT")
                nc.scalar.activation(hT[:], fb[:, 0:D], AF.Copy)
                nc.tensor.matmul(fb[:, D : 2 * D], wrel[:, 0:D], hT[:],
                                 start=True, stop=True)
                nc.tensor.matmul(fb[:, 2 * D : 3 * D], wrel[:, D : 2 * D], hT[:],
                                 start=True, stop=True)
                rl = ffs.tile([P, 2 * D], F32, tag="rl")
                nc.scalar.activation(rl[:], fb[:, D : 3 * D], AF.Relu)
                nc.tensor.matmul(fb[:, 3 * D : 4 * D], rl[:, 0:D], wr0[:],
                                 start=True, stop=False)
                nc.tensor.matmul(fb[:, 3 * D : 4 * D], rl[:, D : 2 * D], wr1[:],
                                 start=False, stop=True)
                nc.vector.tensor_copy(Gch[:, i, :], fb[:, 3 * D : 4 * D])

            # LN2 on (o2 + hl)
            nc.vector.tensor_add(Av, Gv, HLv)
            mu2 = chs.tile([P, CHUNK], F32, tag="mu2")
            nc.vector.tensor_reduce(mu2[:, 0:cw], Av, AX.X, OP.add)
            nc.vector.tensor_scalar_mul(mu2[:, 0:cw], mu2[:, 0:cw], 1.0 / D)
            nc.vector.tensor_tensor(out=Av, in0=Av, in1=bc2(mu2[:, 0:cw]), op=OP.subtract)
            nc.vector.tensor_mul(HLv, Av, Av)
            vv2 = chs.tile([P, CHUNK], F32, tag="vv2")
            nc.vector.tensor_reduce(vv2[:, 0:cw], HLv, AX.X, OP.add)
            nc.vector.tensor_scalar(vv2[:, 0:cw], vv2[:, 0:cw], 1.0 / D, EPS,
                                    OP.mult, OP.add)
            sd2 = chs.tile([P, CHUNK], F32, tag="sd2")
            nc.scalar.activation(sd2[:, 0:cw], vv2[:, 0:cw], AF.Sqrt)
            nc.vector.reciprocal(sd2[:, 0:cw], sd2[:, 0:cw])
            nc.vector.tensor_tensor(out=Av, in0=Av, in1=bc2(sd2[:, 0:cw]), op=OP.mult)
            nc.vector.tensor_tensor(out=Av, in0=Av, in1=bc1(g2), op=OP.mult)
            nc.vector.tensor_tensor(out=Av, in0=Av, in1=bc1(b2), op=OP.add)
            nc.sync.dma_start(
                out_d[c0 * P : (c0 + cw) * P, :].rearrange("(t p) d -> p t d", p=P),
                Av)

        Ach = Gch = HLch = Sch = None
        for t in range(TPC if STAGE > 1 else 0):
            tl = t % CHUNK
            if tl == 0:
                Ach = chp.tile([P, CHUNK, D], F32, tag="A")
                Gch = chp.tile([P, CHUNK, D], F32, tag="G")
                HLch = chp.tile([P, CHUNK, D], F32, tag="HL")
                Sch = chp.tile([P, CHUNK, H], F32, tag="S")
            nbl, nbh = NB_LO[t], NB_HI[t]
            nbt = nbl + nbh
            off = OFFB[t]

            gt = epg.tile([P, NBM, 3 * D], F32, tag="gt")
            idxT = epi.tile([P, NBM * 8], I16, tag="idx")
            nc.sync.dma_start(idxT[:, 0 : nbt * 8], idx_d[:, off * 8 : (off + nbt) * 8])
            stT = epi.tile([P, NBM, P], BF16, tag="stT")
            nc.sync.dma_start(stT[:, 0:nbt, :], stT_d[:, off * P : (off + nbt) * P])

            g1i = nc.gpsimd.dma_gather(
                out_ap=gt[:, 0:nbl, :], in_ap=gall[0:NR2, :],
                idxs_ap=idxT[:, 0 : nbl * 8], num_idxs=nbl * P,
                num_idxs_reg=nreg(nbl * P), elem_size=3 * D, single_packet=False)
            tile.add_dep_helper(g1i.ins, lib_inst.ins, reason="gpsimd lib load")
            if nbh:
                g2i = nc.gpsimd.dma_gather(
                    out_ap=gt[:, nbl:nbt, :], in_ap=gall[SPLIT:NR2, :],
                    idxs_ap=idxT[:, nbl * 8 : nbt * 8], num_idxs=nbh * P,
                    num_idxs_reg=nreg(nbh * P), elem_size=3 * D, single_packet=False)
                tile.add_dep_helper(g2i.ins, lib_inst.ins, reason="gpsimd lib load")

            if STAGE == 2:
                nc.sync.dma_start(out_d[t * P : (t + 1) * P, :], gt[:, 0, 0:D])
                continue

            # st one-hot (f32) in one broadcast compare
            st = eps.tile([P, NBM, P], F32, tag="st")
            nc.vector.tensor_tensor(
                out=st[:, 0:nbt, :],
                in0=iot[:].unsqueeze(1).broadcast_to([P, nbt, P]),
                in1=sdl[:, off : off + nbt].unsqueeze(2).broadcast_to([P, nbt, P]),
                op=OP.is_equal)

            # qg = stT^T @ q_tile  (bf16 matmuls, one per block)
            qg = epq.tile([P, NBM, D], F32, tag="qg")
            for b in range(nbt):
                nc.tensor.matmul(qg[:, b, :], stT[:, b, :], qall[:, t, :],
                                 start=True, stop=True)

            r4e = lambda ap: ap.rearrange("p b (h c) -> p b h c", h=H)
            vs = eps.tile([P, NBM, D + H], F32, tag="vs")
            nc.vector.tensor_tensor(
                out=r4e(vs[:, 0:nbt, 0:D]), in0=r4e(qg[:, 0:nbt, :]),
                in1=r4e(gt[:, 0:nbt, D : 2 * D]), op=OP.mult)
            al = eps.tile([P, NBM, H], F32, tag="al")
            nc.vector.tensor_reduce(al[:, 0:nbt, :], r4e(vs[:, 0:nbt, 0:D]),
                                    AX.X, OP.add)
            nc.scalar.activation(vs[:, 0:nbt, D : D + H], al[:, 0:nbt, :],
                                 AF.Exp, scale=inv_sqrt_c)
            a_b = vs[:, 0:nbt, D : D + H].unsqueeze(3).broadcast_to([P, nbt, H, C])
            nc.vector.tensor_tensor(
                out=r4e(vs[:, 0:nbt, 0:D]), in0=r4e(gt[:, 0:nbt, 2 * D : 3 * D]),
                in1=a_b, op=OP.mult)

            if STAGE == 3:
                po = eps.tile([P, D], F32, tag="po3")
                nc.vector.tensor_copy(po[:], vs[:, 0, 0:D])
                nc.sync.dma_start(out_d[t * P : (t + 1) * P, :], po[:])
                continue

            pa = epab.tile([P, D], F32, tag="pa")
            pb = epab.tile([P, D + H], F32, tag="pb")
            for b in range(nbt):
                nc.tensor.matmul(pa[:], st[:, b, :], gt[:, b, 0:D],
                                 start=(b == 0), stop=(b == nbt - 1))
                nc.tensor.matmul(pb[:], st[:, b, :], vs[:, b, :],
                                 start=(b == 0), stop=(b == nbt - 1))
            nc.scalar.activation(Gch[:, tl, :], pa[:], AF.Copy)
            nc.vector.tensor_copy(Ach[:, tl, :], pb[:, 0:D])
            nc.vector.tensor_copy(Sch[:, tl, :], pb[:, D : D + H])

            if STAGE == 4:
                if tl == 0:
                    nc.sync.dma_start(
                        out_d[t * P : (t + 1) * P, :], Ach[:, tl, :])
                continue

            if tl == CHUNK - 1 or t == TPC - 1:
                emit_post(t - tl, tl + 1, Ach, Gch, HLch, Sch)

    nc.compile()
    return nc


def make_in_maps(meta, percore, weights):
    w = weights
    lw = float(np.asarray(w["local_w"]).reshape(-1)[0])
    rep = lambda v: np.tile(np.asarray(v, np.float32).reshape(1, -1), (P, 1))
    wb = np.asarray(w["w_beta"], np.float32).reshape(-1)
    com = dict(
        W_all=np.hstack([np.asarray(w["w_gcn"], np.float32) * lw,
                         np.asarray(w["wk"], np.float32),
                         np.asarray(w["wv"], np.float32)]).astype(np.float32),
        wq_skip=np.hstack([w["wq"], w["w_skip"]]).astype(np.float32),
        b_cat_rep=rep(np.concatenate([np.asarray(w["bk"]), np.asarray(w["bv"])])),
        bq_rep=rep(w["bq"]),
        b_skip_rep=rep(w["b_skip"]),
        b_gcn_rep=rep(np.asarray(w["b_gcn"], np.float32) * lw),
        g1_rep=rep(w["g1"]), b1_rep=rep(w["b1"]),
        g2_rep=rep(w["g2"]), b2_rep=rep(w["b2"]),
        wba_rep=rep(wb[0:D] + wb[2 * D : 3 * D]),
        wbb_rep=rep(wb[D : 2 * D] - wb[2 * D : 3 * D]),
        w_rel=np.asarray(w["w_rel"], np.float32),
        w_root0=np.asarray(w["w_root"][:D], np.float32),
        w_root1=np.asarray(w["w_root"][D:], np.float32),
        iota_rep=np.tile(np.arange(P, dtype=np.float32).reshape(1, P), (P, 1)),
        ident=np.eye(P, dtype=np.float32),
    )
    in_maps = []
    for c in range(NCORES):
        m = dict(com)
        m["xT"] = percore["xT"][c]
        m["dego"] = percore["dego"][c]
        m["idx_rep"] = percore["idx_rep"][c]
        m["dst_col"] = percore["dst_col"][c]
        m["stT"] = percore["stT"][c]
        in_maps.append(m)
    return in_maps


def _ensure_ntff_hook():
    """Provide antenv.axon_hooks (missing in this image) so bass_utils can
    NTFF-profile through the axon PJRT .so."""
    try:
        from antenv.axon_hooks import get_axon_ntff_profile_hook  # noqa: F401
        return
    except ImportError:
        pass
    import contextlib
    import ctypes
    import sys
    import types

    so_path = "/opt/axon/libaxon_pjrt.so"
    holder = [None]
    mod = types.ModuleType("antenv.axon_hooks")
    mod.set_axon_ntff_profile_hook = lambda h: holder.__setitem__(0, h)
    mod.get_axon_ntff_profile_hook = lambda: holder[0]
    sys.modules["antenv.axon_hooks"] = mod
    try:
        import antenv

        antenv.axon_hooks = mod
    except ImportError:
        pass
    if not os.path.exists(so_path):
        return
    lib = ctypes.CDLL(so_path)
    if not hasattr(lib, "axon_start_nrt_profile"):
        return
    lib.axon_start_nrt_profile.argtypes = [
        ctypes.POINTER(ctypes.c_int64),
        ctypes.c_size_t,
    ]
    lib.axon_start_nrt_profile.restype = ctypes.c_int64
    lib.axon_stop_nrt_profile.argtypes = [ctypes.c_char_p]
    lib.axon_stop_nrt_profile.restype = ctypes.c_int64

    @contextlib.contextmanager
    def _hook(output_dir, device_ids):
        import jax

        jax.devices()
        if device_ids:
            ids = (ctypes.c_int64 * len(device_ids))(*device_ids)
            rc = lib.axon_start_nrt_profile(ids, len(device_ids))
        else:
            rc = lib.axon_start_nrt_profile(None, 0)
        if rc != 0:
            raise RuntimeError(f"axon_start_nrt_profile rc={rc}")
        try:
            yield
        finally:
            n = lib.axon_stop_nrt_profile(str(output_dir).encode())
            print(f"ntff profile: {n} file(s) written to {output_dir}")

    holder[0] = _hook


def run(x, edge_index, weights, trace=False):
    global LAST_EXEC_NS, LAST_RESULTS, LAST_NC
    if trace:
        _ensure_ntff_hook()
    meta, percore = preprocess(x, edge_index)
    gw = float(np.asarray(weights["global_w"]).reshape(-1)[0])
    nc = build_program(meta, gw)
    LAST_NC = nc
    in_maps = make_in_maps(meta, percore, weights)
    res = run_bass_kernel_spmd(nc, in_maps, list(range(NCORES)), trace=trace)
    LAST_EXEC_NS = res.exec_time_ns
    LAST_RESULTS = res
    parts = []
    for c in range(NCORES):
        a, b = c * CORE_NODES, min((c + 1) * CORE_NODES, N)
        parts.append(res.results[c]["out"][: b - a])
    return np.concatenate(parts, axis=0)


def kernel(**inputs):
    x = np.asarray(inputs["x"], dtype=np.float32)
    edge_index = np.asarray(inputs["edge_index"])
    wnames = [
        "w_gcn", "b_gcn", "wq", "bq", "wk", "bk", "wv", "bv", "w_skip", "b_skip",
        "w_beta", "g1", "b1", "g2", "b2", "w_rel", "w_root", "local_w", "global_w",
    ]
    weights = {k: np.asarray(inputs[k], dtype=np.float32) for k in wnames}
    trace = os.environ.get("GPS_TRACE", "0") == "1"
    return run(x, edge_index, weights, trace=trace)


# revision 38
# speedup vs baseline: 1.0070x; 1.0070x over previous
"""Trainium2 Bass kernel for nn_GPSLayer (GCNConv + TransformerConv + FFN).

Strategy (v2):
  - Node phase SPLIT across the 8 cores: core c computes [dinv*x@Wgcn | x@Wk+bk
    | x@Wv+bv] for its 1/8 node slice only, then an AllGather (Shared output)
    assembles the full [NR2, 3D] gather-source in HBM.  q = x@Wq+bq and
    x_skip = x@Wskip+bskip stay core-local in SBUF (q in bf16).
  - Edge phase: dst nodes sharded by contiguous 128-ranges; per dst-tile,
    dma_gather the [E_t, 384] payload rows (lo/hi halves for int16 indexing;
    padding slots use idx=-1 which the Q7 ucode strips for free).
  - q[dst] per edge is NOT gathered: qg = stT^T @ q_tile on the PE, where
    stT (one-hot, bf16) is host-precomputed and DMA'd.  st (f32 one-hot for
    the segment-sum matmuls) is built on-chip with one broadcast is_eq.
  - Segment-sum via one-hot matmuls into PSUM (pa=gcn, pb=[attn*v|sum exp]).
  - Post phase (softmax normalize, beta-gate, LN, FFN, LN) runs batched over
    chunks of 13 dst tiles with [128, chunk*128] vector ops.
"""

import math
import os
from contextlib import ExitStack

import numpy as np
import ml_dtypes

import concourse.bacc as bacc
import concourse.bass as bass
import concourse.mybir as mybir
import concourse.tile as tile
from concourse.bass_utils import run_bass_kernel_spmd
from concourse.library_config import mlp as MLP_LIB

F32 = mybir.dt.float32
BF16 = mybir.dt.bfloat16
I16 = mybir.dt.int16
AX = mybir.AxisListType
OP = mybir.AluOpType
AF = mybir.ActivationFunctionType

N = 50000
D = 128
H = 4
C = D // H
P = 128
NCORES = 8
TPC = 49                      # dst/node tiles per core
CORE_NODES = TPC * P          # 6272
NR2 = NCORES * CORE_NODES     # 50176 padded rows
SPLIT = 32768
EPS = 1e-5
PAD_DL = 200.0
GCOLS = 320                   # payload row: [gcn f32 | v f32 | k bf16] = 1280B
AGJ = [0, 1536, 3072, 4608, 6272]   # AllGather chunk bounds (per-core rows)
CHUNKS = [12, 12, 12, 8, 5]         # post-phase batch widths (sum = TPC)
CHUNKW = max(CHUNKS)

LAST_EXEC_NS = None
LAST_RESULTS = None
LAST_NC = None


def _ceil64(v):
    return max(64, -(-int(v) // 64) * 64)


def preprocess(x, edge_index):
    """Host-side: shard+sort edges, build gather indices into the permuted
    (AG-chunk-interleaved) gall layout, st^T one-hot (bf16), per-core
    transposed x slices and degree columns."""
    src = np.asarray(edge_index[0]).astype(np.int64)
    dst = np.asarray(edge_index[1]).astype(np.int64)
    E = src.shape[0]
    deg = np.bincount(dst, minlength=N).astype(np.float32)

    # gall row permutation: chunk-major [rank-interleaved per AG chunk]
    nn = np.arange(NR2, dtype=np.int64)
    rr, jj_ = nn // CORE_NODES, nn % CORE_NODES
    agj = np.asarray(AGJ, dtype=np.int64)
    ch = np.searchsorted(agj, jj_, side="right") - 1
    ch_rows = agj[ch + 1] - agj[ch]
    perm = NCORES * agj[ch] + rr * ch_rows + (jj_ - agj[ch])

    core = dst // CORE_NODES
    tl = (dst % CORE_NODES) // P
    dl = (dst % P).astype(np.int64)
    ps = perm[src]
    hi = (ps >= SPLIT).astype(np.int64)

    key = (core * TPC + tl) * 2 + hi
    ngroups = NCORES * TPC * 2
    counts = np.bincount(key, minlength=ngroups).reshape(NCORES, TPC, 2)
    mx = counts.max(axis=0)                     # [TPC, 2] max over cores

    # uniform-across-cores block counts and exact (64-rounded) gather counts
    NCNT_LO = np.array([_ceil64(m) for m in mx[:, 0]], dtype=np.int64)
    NCNT_HI = np.array([_ceil64(m) if m > 0 else 0 for m in mx[:, 1]], dtype=np.int64)
    NB_LO = -(-NCNT_LO // P)
    NB_HI = -(-NCNT_HI // P)
    NBT = NB_LO + NB_HI
    OFFB = np.concatenate([[0], np.cumsum(NBT)])[:-1]
    TB = int(NBT.sum())

    lo_start = OFFB * P
    hi_start = (OFFB + NB_LO) * P
    group_base = np.stack([lo_start, hi_start], axis=1)  # [TPC, 2]

    order = np.argsort(key, kind="stable")
    group_start_idx = np.concatenate([[0], np.cumsum(counts.reshape(-1))])[:-1]
    rank = np.empty(E, dtype=np.int64)
    rank[order] = np.arange(E) - group_start_idx[key[order]]

    pos = group_base[tl, hi] + rank
    srcv = np.where(hi == 1, ps - SPLIT, ps).astype(np.int16)

    # pad slots gather row 0 (the deployed Q7 ucode mishandles -1 sentinels);
    # slots past the exact64 count are never gathered (stale, st-masked).
    idx_all = np.zeros((NCORES, TB * P), dtype=np.int16)
    dst_all = np.full((NCORES, TB * P), PAD_DL, dtype=np.float32)
    flat = core * (TB * P) + pos
    idx_all.reshape(-1)[flat] = srcv
    dst_all.reshape(-1)[flat] = dl.astype(np.float32)

    # wrapped int16 index layout [16, TB*8] replicated to 128 partitions
    idx_wrap = np.ascontiguousarray(idx_all.reshape(NCORES, TB * 8, 16).transpose(0, 2, 1))
    idx_rep = np.tile(idx_wrap, (1, 8, 1))                       # [NCORES, 128, TB*8]
    dst_col = np.ascontiguousarray(dst_all.reshape(NCORES, TB, P).transpose(0, 2, 1))

    # stT one-hot bf16: stT[c, j, b*128+e] = (dst_all[c, b*128+e] == j)
    jj = np.arange(P, dtype=np.float32)[:, None]
    stT = np.empty((NCORES, P, TB * P), dtype=ml_dtypes.bfloat16)
    for c in range(NCORES):
        stT[c] = (dst_all[c][None, :] == jj).astype(ml_dtypes.bfloat16)

    # per-core transposed x slice + degree column
    x = np.asarray(x, dtype=np.float32)
    x_pad = np.zeros((NR2, D), dtype=np.float32)
    x_pad[:N] = x
    deg_pad = np.ones(NR2, dtype=np.float32)
    deg_pad[:N] = deg
    xT = np.empty((NCORES, D, CORE_NODES), dtype=np.float32)
    dego = np.empty((NCORES, P, TPC), dtype=np.float32)
    for c in range(NCORES):
        a, b = c * CORE_NODES, (c + 1) * CORE_NODES
        xT[c] = np.ascontiguousarray(x_pad[a:b].T)
        dego[c] = np.ascontiguousarray(deg_pad[a:b].reshape(TPC, P).T)

    meta = dict(NB_LO=NB_LO.tolist(), NB_HI=NB_HI.tolist(), OFFB=OFFB.tolist(),
                NCNT_LO=NCNT_LO.tolist(), NCNT_HI=NCNT_HI.tolist(),
                TB=TB, NBT_MAX=int(NBT.max()))
    percore = dict(idx_rep=idx_rep, dst_col=dst_col, stT=stT, xT=xT, dego=dego)
    return meta, percore


def build_program(meta, gw_scalar):
    NB_LO, NB_HI, OFFB = meta["NB_LO"], meta["NB_HI"], meta["OFFB"]
    NCNT_LO, NCNT_HI = meta["NCNT_LO"], meta["NCNT_HI"]
    TB, NBM = meta["TB"], meta["NBT_MAX"]

    nc = bacc.Bacc("TRN2", target_bir_lowering=False, debug=False, num_devices=NCORES)

    def inp(name, shape, dt=F32):
        return nc.dram_tensor(name, shape, dt, kind="ExternalInput").ap()

    xT_d = inp("xT", [D, CORE_NODES])
    dego_d = inp("dego", [P, TPC])
    wall_d = inp("W_all", [D, 3 * D])
    wqs_d = inp("wq_skip", [D, 2 * D])
    bv_d = inp("bv_rep", [P, D])
    bk_d = inp("bk_rep", [P, D])
    bq_d = inp("bq_rep", [P, D])
    bskip_d = inp("b_skip_rep", [P, D])
    bgcn_d = inp("b_gcn_rep", [P, D])
    g1_d = inp("g1_rep", [P, D])
    b1_d = inp("b1_rep", [P, D])
    g2_d = inp("g2_rep", [P, D])
    b2_d = inp("b2_rep", [P, D])
    wba_d = inp("wba_rep", [P, D])
    wbb_d = inp("wbb_rep", [P, D])
    wrel_d = inp("w_rel", [D, 2 * D])
    wr0_d = inp("w_root0", [D, D])
    wr1_d = inp("w_root1", [D, D])
    iot_d = inp("iota_rep", [P, P])
    idn_d = inp("ident", [P, P])
    idx_d = inp("idx_rep", [P, TB * 8], I16)
    sdl_d = inp("dst_col", [P, TB])
    stT_d = inp("stT", [P, TB * P], BF16)

    gparts = [
        nc.dram_tensor(f"gpart{i}", [AGJ[i + 1] - AGJ[i], GCOLS], F32).ap()
        for i in range(len(AGJ) - 1)
    ]
    gall = nc.dram_tensor("gall", [NR2, GCOLS], F32, addr_space="Shared").ap()
    out_d = nc.dram_tensor("out", [CORE_NODES, D], F32, kind="ExternalOutput").ap()

    inv_sqrt_c = 1.0 / math.sqrt(C)

    with tile.TileContext(nc) as tc, ExitStack() as ctx:
        lib_inst = nc.gpsimd.load_library(MLP_LIB)
        cp = ctx.enter_context(tc.tile_pool(name="const", bufs=1))

        def ctile(src_ap, shape, dt=F32, tag=None):
            t = cp.tile(shape, dt, tag=tag)
            nc.sync.dma_start(t[:], src_ap)
            return t

        wall = ctile(wall_d, [D, 3 * D], tag="wall")
        wqs = ctile(wqs_d, [D, 2 * D], tag="wqs")
        bv = ctile(bv_d, [P, D], tag="bv")
        bk = ctile(bk_d, [P, D], tag="bk")
        bq = ctile(bq_d, [P, D], tag="bq")
        bskip = ctile(bskip_d, [P, D], tag="bskip")
        bgcn = ctile(bgcn_d, [P, D], tag="bgcn")
        g1 = ctile(g1_d, [P, D], tag="g1")
        b1 = ctile(b1_d, [P, D], tag="b1")
        g2 = ctile(g2_d, [P, D], tag="g2")
        b2 = ctile(b2_d, [P, D], tag="b2")
        wba = ctile(wba_d, [P, D], tag="wba")
        wbb = ctile(wbb_d, [P, D], tag="wbb")
        wrel = ctile(wrel_d, [D, 2 * D], tag="wrel")
        wr0 = ctile(wr0_d, [D, D], tag="wr0")
        wr1 = ctile(wr1_d, [D, D], tag="wr1")
        iot = ctile(iot_d, [P, P], tag="iot")
        idn = ctile(idn_d, [P, P], tag="idn")
        sdl = ctile(sdl_d, [P, TB], tag="sdl")
        sdego = ctile(dego_d, [P, TPC], tag="sdego")

        dinvo = cp.tile([P, TPC], F32, tag="dinvo")
        with tc.tile_pool(name="setup", bufs=1) as sp:
            t1 = sp.tile([P, TPC], F32, tag="din1")
            t2 = sp.tile([P, TPC], F32, tag="din2")
            nc.vector.tensor_scalar_max(t1[:], sdego[:], 1.0)
            nc.scalar.activation(t2[:], t1[:], AF.Sqrt)
            nc.vector.reciprocal(t1[:], t2[:])
            nc.vector.tensor_scalar(t2[:], sdego[:], 0.0, None, OP.is_gt)
            nc.vector.tensor_mul(dinvo[:], t1[:], t2[:])

        pers = ctx.enter_context(tc.tile_pool(name="pers", bufs=1))
        qall = pers.tile([P, TPC, D], BF16, tag="qall")
        xrs = pers.tile([P, TPC, D], F32, tag="xrs")



        # ---------------- node phase (own 1/8 slice only) ----------------
        # [gcn f32 | v f32 | k bf16] payload rows; AllGather chunk c fires as
        # soon as its node tiles are written so the collectives overlap the
        # rest of the node phase.
        ag_ends = {AGJ[i + 1] // P - 1: i for i in range(len(AGJ) - 1)}
        with ExitStack() as nscope:
            npx = nscope.enter_context(tc.tile_pool(name="npx", bufs=3))
            npo = nscope.enter_context(tc.tile_pool(name="npo", bufs=3))
            ppsn = nscope.enter_context(tc.tile_pool(name="ppsn", bufs=2, space="PSUM"))
            for t in range(TPC):
                xt = npx.tile([D, P], F32, tag="xt")
                nc.sync.dma_start(xt[:], xT_d[:, t * P : (t + 1) * P])
                pm1 = ppsn.tile([P, 3 * D], F32, tag="pm1")
                nc.tensor.matmul(pm1[:], xt[:], wall[:], start=True, stop=True)
                pm2 = ppsn.tile([P, 2 * D], F32, tag="pm2")
                nc.tensor.matmul(pm2[:], xt[:], wqs[:], start=True, stop=True)
                go = npo.tile([P, GCOLS], F32, tag="go")
                nc.scalar.activation(go[:, 0:D], pm1[:, 0:D], AF.Copy,
                                     scale=dinvo[:, t : t + 1])
                nc.vector.tensor_add(go[:, D : 2 * D], pm1[:, D : 2 * D], bv[:])
                nc.vector.tensor_add(
                    go[:, 2 * D : GCOLS].bitcast(BF16), pm1[:, 2 * D : 3 * D], bk[:])
                ci = np.searchsorted(AGJ, t * P, side="right") - 1
                r0 = t * P - AGJ[ci]
                nc.sync.dma_start(gparts[ci][r0 : r0 + P, :], go[:])
                nc.vector.tensor_add(qall[:, t, :], pm2[:, 0:D], bq[:])
                nc.vector.tensor_add(xrs[:, t, :], pm2[:, D : 2 * D], bskip[:])
                if t in ag_ends:
                    i = ag_ends[t]
                    nc.gpsimd.collective_compute(
                        "AllGather", OP.bypass,
                        replica_groups=[list(range(NCORES))],
                        ins=[gparts[i][:, :]],
                        outs=[gall[NCORES * AGJ[i] : NCORES * AGJ[i + 1], :]],
                    )

        STAGE = int(os.environ.get("GPS_STAGE", "9"))
        if STAGE == 1:
            with tc.tile_pool(name="dbg", bufs=1) as dbg:
                for r in range(4):
                    dt_ = dbg.tile([P, GCOLS], F32, tag="dbg")
                    nc.sync.dma_start(dt_[:], gall[r * 12288 : r * 12288 + P, :])
                    nc.sync.dma_start(out_d[r * P : (r + 1) * P, :], dt_[:, 0:D])

        # ---------------- edge phase ----------------
        epg = ctx.enter_context(tc.tile_pool(name="epg", bufs=4))
        epi = ctx.enter_context(tc.tile_pool(name="epi", bufs=3))
        eps = ctx.enter_context(tc.tile_pool(name="eps", bufs=2))
        epq = ctx.enter_context(tc.tile_pool(name="epq", bufs=1, space="PSUM"))
        epab = ctx.enter_context(tc.tile_pool(name="epab", bufs=2, space="PSUM"))
        chp = ctx.enter_context(tc.tile_pool(name="chp", bufs=2))
        chs = ctx.enter_context(tc.tile_pool(name="chs", bufs=2))
        ffp = ctx.enter_context(tc.tile_pool(name="ffp", bufs=1, space="PSUM"))
        ffs = ctx.enter_context(tc.tile_pool(name="ffs", bufs=2))

        # zero the gather buffers once: slots past the exact64 gather count
        # keep stale bytes; first rounds must not contain NaN bit patterns.
        for _ in range(4):
            z = epg.tile([P, NBM, GCOLS], F32, tag="gt")
            nc.vector.memset(z[:], 0.0)

        reg_cache = {}

        def nreg(val):
            if val not in reg_cache:
                r = nc.gpsimd.alloc_register(f"ni_{val}")
                nc.gpsimd.reg_mov(r, val)
                reg_cache[val] = r
            return reg_cache[val]

        def emit_post(c0, cw, Ach, Gch, HLch, Sch):
            Av = Ach[:, 0:cw, :]
            Gv = Gch[:, 0:cw, :]
            HLv = HLch[:, 0:cw, :]
            Sv = Sch[:, 0:cw, :]
            r4 = lambda ap: ap.rearrange("p t (h c) -> p t h c", h=H)
            bc1 = lambda t_: t_[:].unsqueeze(1).broadcast_to([P, cw, D])
            bc2 = lambda ap: ap.unsqueeze(2).broadcast_to([P, cw, D])

            if STAGE == 7:
                nc.sync.dma_start(
                    out_d[c0 * P : (c0 + cw) * P, :].rearrange("(t p) d -> p t d", p=P),
                    xrs[:, c0 : c0 + cw, :])
                return

            # softmax normalize + gcn degree scale
            nc.vector.tensor_scalar_max(Sv, Sv, 1e-16)
            nc.vector.reciprocal(Sv, Sv)
            nc.vector.tensor_tensor(
                out=r4(Av), in0=r4(Av),
                in1=Sv.unsqueeze(3).broadcast_to([P, cw, H, C]), op=OP.mult)
            if STAGE == 8:
                nc.sync.dma_start(
                    out_d[c0 * P : (c0 + cw) * P, :].rearrange("(t p) d -> p t d", p=P),
                    Av)
                return
            nc.vector.tensor_tensor(
                out=Gv, in0=Gv, in1=bc2(dinvo[:, c0 : c0 + cw]), op=OP.mult)
            nc.vector.tensor_tensor(out=Gv, in0=Gv, in1=bc1(bgcn), op=OP.add)
            if STAGE == 10:
                nc.sync.dma_start(
                    out_d[c0 * P : (c0 + cw) * P, :].rearrange("(t p) d -> p t d", p=P),
                    Gv)
                return

            # beta gate
            Xv = xrs[:, c0 : c0 + cw, :]
            r1 = chs.tile([P, CHUNKW], F32, tag="r1")
            r2 = chs.tile([P, CHUNKW], F32, tag="r2")
            nc.vector.tensor_tensor(out=HLv, in0=Av, in1=bc1(wba), op=OP.mult)
            nc.vector.tensor_reduce(r1[:, 0:cw], HLv, AX.X, OP.add)
            nc.vector.tensor_tensor(out=HLv, in0=Xv, in1=bc1(wbb), op=OP.mult)
            nc.vector.tensor_reduce(r2[:, 0:cw], HLv, AX.X, OP.add)
            nc.vector.tensor_add(r1[:, 0:cw], r1[:, 0:cw], r2[:, 0:cw])
            bet = chs.tile([P, CHUNKW], F32, tag="bet")
            nc.scalar.activation(bet[:, 0:cw], r1[:, 0:cw], AF.Sigmoid)
            nc.vector.tensor_sub(HLv, Xv, Av)
            nc.vector.tensor_tensor(out=HLv, in0=HLv, in1=bc2(bet[:, 0:cw]), op=OP.mult)
            nc.vector.tensor_add(HLv, HLv, Av)
            if gw_scalar != 1.0:
                nc.vector.tensor_scalar_mul(HLv, HLv, float(gw_scalar))
            nc.vector.tensor_add(Gv, Gv, HLv)      # h
            if STAGE == 5:
                nc.sync.dma_start(
                    out_d[c0 * P : (c0 + cw) * P, :].rearrange("(t p) d -> p t d", p=P),
                    Gv)
                return

            # LN1  (LN(2h) == LN(h))
            mu = chs.tile([P, CHUNKW], F32, tag="mu")
            nc.vector.tensor_reduce(mu[:, 0:cw], Gv, AX.X, OP.add)
            nc.vector.tensor_scalar_mul(mu[:, 0:cw], mu[:, 0:cw], 1.0 / D)
            nc.vector.tensor_tensor(out=Gv, in0=Gv, in1=bc2(mu[:, 0:cw]), op=OP.subtract)
            nc.vector.tensor_mul(HLv, Gv, Gv)
            vv = chs.tile([P, CHUNKW], F32, tag="vv")
            nc.vector.tensor_reduce(vv[:, 0:cw], HLv, AX.X, OP.add)
            nc.vector.tensor_scalar(vv[:, 0:cw], vv[:, 0:cw], 1.0 / D, EPS,
                                    OP.mult, OP.add)
            sd = chs.tile([P, CHUNKW], F32, tag="sd")
            nc.scalar.activation(sd[:, 0:cw], vv[:, 0:cw], AF.Sqrt)
            nc.vector.reciprocal(sd[:, 0:cw], sd[:, 0:cw])
            nc.vector.tensor_tensor(out=HLv, in0=Gv, in1=bc2(sd[:, 0:cw]), op=OP.mult)
            nc.vector.tensor_tensor(out=HLv, in0=HLv, in1=bc1(g1), op=OP.mult)
            nc.vector.tensor_tensor(out=HLv, in0=HLv, in1=bc1(b1), op=OP.add)  # hl
            if STAGE == 6:
                nc.sync.dma_start(
                    out_d[c0 * P : (c0 + cw) * P, :].rearrange("(t p) d -> p t d", p=P),
                    HLv)
                return

            # FFN per tile; o2 overwrites Gch (xc dead).  pT/o1/o2 packed into
            # one PSUM bank: [0:D]=pT, [D:3D]=o1, [3D:4D]=o2.
            for i in range(cw):
                fb = ffp.tile([P, 4 * D], F32, tag="fb")
                nc.tensor.transpose(fb[:, 0:D], HLch[:, i, :], idn[:])
                hT = ffs.tile([P, D], F32, tag="hT")
                nc.scalar.activation(hT[:], fb[:, 0:D], AF.Copy)
                nc.tensor.matmul(fb[:, D : 2 * D], wrel[:, 0:D], hT[:],
                                 start=True, stop=True)
                nc.tensor.matmul(fb[:, 2 * D : 3 * D], wrel[:, D : 2 * D], hT[:],
                                 start=True, stop=True)
                rl = ffs.tile([P, 2 * D], F32, tag="rl")
                nc.scalar.activation(rl[:], fb[:, D : 3 * D], AF.Relu)
                nc.tensor.matmul(fb[:, 3 * D : 4 * D], rl[:, 0:D], wr0[:],
                                 start=True, stop=False)
                nc.tensor.matmul(fb[:, 3 * D : 4 * D], rl[:, D : 2 * D], wr1[:],
                                 start=False, stop=True)
                nc.vector.tensor_copy(Gch[:, i, :], fb[:, 3 * D : 4 * D])

            # LN2 on (o2 + hl)
            nc.vector.tensor_add(Av, Gv, HLv)
            mu2 = chs.tile([P, CHUNKW], F32, tag="mu2")
            nc.vector.tensor_reduce(mu2[:, 0:cw], Av, AX.X, OP.add)
            nc.vector.tensor_scalar_mul(mu2[:, 0:cw], mu2[:, 0:cw], 1.0 / D)
            nc.vector.tensor_tensor(out=Av, in0=Av, in1=bc2(mu2[:, 0:cw]), op=OP.subtract)
            nc.vector.tensor_mul(HLv, Av, Av)
            vv2 = chs.tile([P, CHUNKW], F32, tag="vv2")
            nc.vector.tensor_reduce(vv2[:, 0:cw], HLv, AX.X, OP.add)
            nc.vector.tensor_scalar(vv2[:, 0:cw], vv2[:, 0:cw], 1.0 / D, EPS,
                                    OP.mult, OP.add)
            sd2 = chs.tile([P, CHUNKW], F32, tag="sd2")
            nc.scalar.activation(sd2[:, 0:cw], vv2[:, 0:cw], AF.Sqrt)
            nc.vector.reciprocal(sd2[:, 0:cw], sd2[:, 0:cw])
            nc.vector.tensor_tensor(out=Av, in0=Av, in1=bc2(sd2[:, 0:cw]), op=OP.mult)
            nc.vector.tensor_tensor(out=Av, in0=Av, in1=bc1(g2), op=OP.mult)
            nc.vector.tensor_tensor(out=Av, in0=Av, in1=bc1(b2), op=OP.add)
            nc.sync.dma_start(
                out_d[c0 * P : (c0 + cw) * P, :].rearrange("(t p) d -> p t d", p=P),
                Av)

        chunk_bounds = list(np.cumsum([0] + CHUNKS))
        chunk_start = {int(a): int(b - a) for a, b in zip(chunk_bounds[:-1], chunk_bounds[1:])}

        Ach = Gch = HLch = Sch = None
        c0 = cw = 0
        for t in range(TPC if STAGE > 1 else 0):
            if t in chunk_start:
                c0, cw = t, chunk_start[t]
                Ach = chp.tile([P, CHUNKW, D], F32, tag="A")
                Gch = chp.tile([P, CHUNKW, D], F32, tag="G")
                HLch = chp.tile([P, CHUNKW, D], F32, tag="HL")
                Sch = chp.tile([P, CHUNKW, H], F32, tag="S")
            tl = t - c0
            nbl, nbh = NB_LO[t], NB_HI[t]
            nbt = nbl + nbh
            off = OFFB[t]

            gt = epg.tile([P, NBM, GCOLS], F32, tag="gt")
            idxT = epi.tile([P, NBM * 8], I16, tag="idx")
            nc.sync.dma_start(idxT[:, 0 : nbt * 8], idx_d[:, off * 8 : (off + nbt) * 8])
            stT = epi.tile([P, NBM, P], BF16, tag="stT")
            nc.sync.dma_start(stT[:, 0:nbt, :], stT_d[:, off * P : (off + nbt) * P])

            g1i = nc.gpsimd.dma_gather(
                out_ap=gt[:, 0:nbl, :], in_ap=gall[0:NR2, :],
                idxs_ap=idxT[:, 0 : nbl * 8], num_idxs=NCNT_LO[t],
                num_idxs_reg=nreg(NCNT_LO[t]), elem_size=GCOLS, single_packet=False)
            tile.add_dep_helper(g1i.ins, lib_inst.ins, reason="gpsimd lib load")
            if nbh:
                g2i = nc.gpsimd.dma_gather(
                    out_ap=gt[:, nbl:nbt, :], in_ap=gall[SPLIT:NR2, :],
                    idxs_ap=idxT[:, nbl * 8 : nbt * 8], num_idxs=NCNT_HI[t],
                    num_idxs_reg=nreg(NCNT_HI[t]), elem_size=GCOLS, single_packet=False)
                tile.add_dep_helper(g2i.ins, lib_inst.ins, reason="gpsimd lib load")

            if STAGE == 2:
                nc.sync.dma_start(out_d[t * P : (t + 1) * P, :], gt[:, 0, 0:D])
                continue

            # st one-hot (f32) in one broadcast compare
            st = eps.tile([P, NBM, P], F32, tag="st")
            nc.vector.tensor_tensor(
                out=st[:, 0:nbt, :],
                in0=iot[:].unsqueeze(1).broadcast_to([P, nbt, P]),
                in1=sdl[:, off : off + nbt].unsqueeze(2).broadcast_to([P, nbt, P]),
                op=OP.is_equal)

            # qg = stT^T @ q_tile  (bf16 matmuls, one per block)
            qg = epq.tile([P, NBM, D], F32, tag="qg")
            for b in range(nbt):
                nc.tensor.matmul(qg[:, b, :], stT[:, b, :], qall[:, t, :],
                                 start=True, stop=True)

            r4e = lambda ap: ap.rearrange("p b (h c) -> p b h c", h=H)
            kview = gt[:, 0:nbt, 2 * D : GCOLS].bitcast(BF16)
            vs = eps.tile([P, NBM, D + H], F32, tag="vs")
            nc.vector.tensor_tensor(
                out=r4e(vs[:, 0:nbt, 0:D]), in0=r4e(qg[:, 0:nbt, :]),
                in1=r4e(kview), op=OP.mult)
            al = eps.tile([P, NBM, H], F32, tag="al")
            nc.vector.tensor_reduce(al[:, 0:nbt, :], r4e(vs[:, 0:nbt, 0:D]),
                                    AX.X, OP.add)
            nc.scalar.activation(vs[:, 0:nbt, D : D + H], al[:, 0:nbt, :],
                                 AF.Exp, scale=inv_sqrt_c)
            a_b = vs[:, 0:nbt, D : D + H].unsqueeze(3).broadcast_to([P, nbt, H, C])
            nc.vector.tensor_tensor(
                out=r4e(vs[:, 0:nbt, 0:D]), in0=r4e(gt[:, 0:nbt, D : 2 * D]),
                in1=a_b, op=OP.mult)

            if STAGE == 3:
                po = eps.tile([P, D], F32, tag="po3")
                nc.vector.tensor_copy(po[:], vs[:, 0, 0:D])
                nc.sync.dma_start(out_d[t * P : (t + 1) * P, :], po[:])
                continue

            pa = epab.tile([P, D], F32, tag="pa")
            pb = epab.tile([P, D + H], F32, tag="pb")
            for b in range(nbt):
                nc.tensor.matmul(pa[:], st[:, b, :], gt[:, b, 0:D],
                                 start=(b == 0), stop=(b == nbt - 1))
                nc.tensor.matmul(pb[:], st[:, b, :], vs[:, b, :],
                                 start=(b == 0), stop=(b == nbt - 1))
            nc.scalar.activation(Gch[:, tl, :], pa[:], AF.Copy)
            nc.vector.tensor_copy(Ach[:, tl, :], pb[:, 0:D])
            nc.vector.tensor_copy(Sch[:, tl, :], pb[:, D : D + H])

            if STAGE == 4:
                if tl == 0:
                    nc.sync.dma_start(
                        out_d[t * P : (t + 1) * P, :], Ach[:, tl, :])
                continue

            if tl == cw - 1:
                emit_post(c0, cw, Ach, Gch, HLch, Sch)

    nc.compile()
    return nc


def make_in_maps(meta, percore, weights):
    w = weights
    lw = float(np.asarray(w["local_w"]).reshape(-1)[0])
    rep = lambda v: np.tile(np.asarray(v, np.float32).reshape(1, -1), (P, 1))
    wb = np.asarray(w["w_beta"], np.float32).reshape(-1)
    com = dict(
        W_all=np.hstack([np.asarray(w["w_gcn"], np.float32) * lw,
                         np.asarray(w["wv"], np.float32),
                         np.asarray(w["wk"], np.float32)]).astype(np.float32),
        wq_skip=np.hstack([w["wq"], w["w_skip"]]).astype(np.float32),
        bv_rep=rep(w["bv"]),
        bk_rep=rep(w["bk"]),
        bq_rep=rep(w["bq"]),
        b_skip_rep=rep(w["b_skip"]),
        b_gcn_rep=rep(np.asarray(w["b_gcn"], np.float32) * lw),
        g1_rep=rep(w["g1"]), b1_rep=rep(w["b1"]),
        g2_rep=rep(w["g2"]), b2_rep=rep(w["b2"]),
        wba_rep=rep(wb[0:D] + wb[2 * D : 3 * D]),
        wbb_rep=rep(wb[D : 2 * D] - wb[2 * D : 3 * D]),
        w_rel=np.asarray(w["w_rel"], np.float32),
        w_root0=np.asarray(w["w_root"][:D], np.float32),
        w_root1=np.asarray(w["w_root"][D:], np.float32),
        iota_rep=np.tile(np.arange(P, dtype=np.float32).reshape(1, P), (P, 1)),
        ident=np.eye(P, dtype=np.float32),
    )
    in_maps = []
    for c in range(NCORES):
        m = dict(com)
        m["xT"] = percore["xT"][c]
        m["dego"] = percore["dego"][c]
        m["idx_rep"] = percore["idx_rep"][c]
        m["dst_col"] = percore["dst_col"][c]
        m["stT"] = percore["stT"][c]
        in_maps.append(m)
    return in_maps


def _ensure_ntff_hook():
    """Provide antenv.axon_hooks (missing in this image) so bass_utils can
    NTFF-profile through the axon PJRT .so."""
    try:
        from antenv.axon_hooks import get_axon_ntff_profile_hook  # noqa: F401
        return
    except ImportError:
        pass
    import contextlib
    import ctypes
    import sys
    import types

    so_path = "/opt/axon/libaxon_pjrt.so"
    holder = [None]
    mod = types.ModuleType("antenv.axon_hooks")
    mod.set_axon_ntff_profile_hook = lambda h: holder.__setitem__(0, h)
    mod.get_axon_ntff_profile_hook = lambda: holder[0]
    sys.modules["antenv.axon_hooks"] = mod
    try:
        import antenv

        antenv.axon_hooks = mod
    except ImportError:
        pass
    if not os.path.exists(so_path):
        return
    lib = ctypes.CDLL(so_path)
    if not hasattr(lib, "axon_start_nrt_profile"):
        return
    lib.axon_start_nrt_profile.argtypes = [
        ctypes.POINTER(ctypes.c_int64),
        ctypes.c_size_t,
    ]
    lib.axon_start_nrt_profile.restype = ctypes.c_int64
    lib.axon_stop_nrt_profile.argtypes = [ctypes.c_char_p]
    lib.axon_stop_nrt_profile.restype = ctypes.c_int64

    @contextlib.contextmanager
    def _hook(output_dir, device_ids):
        import jax

        jax.devices()
        if device_ids:
            ids = (ctypes.c_int64 * len(device_ids))(*device_ids)
            rc = lib.axon_start_nrt_profile(ids, len(device_ids))
        else:
            rc = lib.axon_start_nrt_profile(None, 0)
        if rc != 0:
            raise RuntimeError(f"axon_start_nrt_profile rc={rc}")
        try:
            yield
        finally:
            n = lib.axon_stop_nrt_profile(str(output_dir).encode())
            print(f"ntff profile: {n} file(s) written to {output_dir}")

    holder[0] = _hook


def run(x, edge_index, weights, trace=False):
    global LAST_EXEC_NS, LAST_RESULTS, LAST_NC
    if trace:
        _ensure_ntff_hook()
    meta, percore = preprocess(x, edge_index)
    gw = float(np.asarray(weights["global_w"]).reshape(-1)[0])
    nc = build_program(meta, gw)
    LAST_NC = nc
    in_maps = make_in_maps(meta, percore, weights)
    res = run_bass_kernel_spmd(nc, in_maps, list(range(NCORES)), trace=trace)
    LAST_EXEC_NS = res.exec_time_ns
    LAST_RESULTS = res
    parts = []
    for c in range(NCORES):
        a, b = c * CORE_NODES, min((c + 1) * CORE_NODES, N)
        parts.append(res.results[c]["out"][: b - a])
    return np.concatenate(parts, axis=0)


def kernel(**inputs):
    x = np.asarray(inputs["x"], dtype=np.float32)
    edge_index = np.asarray(inputs["edge_index"])
    wnames = [
        "w_gcn", "b_gcn", "wq", "bq", "wk", "bk", "wv", "bv", "w_skip", "b_skip",
        "w_beta", "g1", "b1", "g2", "b2", "w_rel", "w_root", "local_w", "global_w",
    ]
    weights = {k: np.asarray(inputs[k], dtype=np.float32) for k in wnames}
    trace = os.environ.get("GPS_TRACE", "0") == "1"
    return run(x, edge_index, weights, trace=trace)
